# revision 21
# baseline (speedup 1.0000x reference)
"""Distributed Trainium2 kernel for nn_ApaBlock (8 NeuronCores, data-parallel).

Transposed dataflow (per core, batch shard of 256 rows), no PE transposes:
  Z^T = relu(W1^T @ X^T + b1)              [h, b] via lhsT=W1 (natural)
  W[p,(q,b)] = Z^T[q,b]                    partition-broadcast table, built
                                           once via ones-matmuls + evac
  scan over 8 ranks:
    M_q[p,b] = Zi^T[p,b] * Z^T[q,b]        DVE tensor_tensor bf16 SBUF (2x)
    G^T[k,b] += P_q^T @ M_q                128 accumulating matmuls (N=256)
    stats via ACT accum_out during PSUM evac; 1KB AllGather (sync-BN);
    Zi+1^T = a*G^T + c on ACT             (per-partition scale+bias)
  Y-BN via closed-form global sums piggybacked on the rank-7 sync.
  out^T = relu(relu(W3^T@Ybn^T+b3) + relu(W2^T@X^T+b2)); host transposes.

vs baseline: TensorE streaming work halved (no identity-matmul q-reduction),
DVE scale moved from 1x PSUM-sourced to 2x SBUF bf16, zero PE transposes.
"""

import os
import sys
import types

if "/opt/trn_rl_repo" not in sys.path:
    sys.path.insert(0, "/opt/trn_rl_repo")

import numpy as np
import ml_dtypes

N_CORES = 8
B, IN, H, OUT, RANK = 2048, 256, 128, 128, 8
BS = B // N_CORES  # 256 rows per core
EPS = 1e-5
QK = H * H  # 16384
NQC = 16            # q values per M chunk
NCH = H // NQC      # 8 chunks per rank
PCH = 4             # P dma split per rank

_cache = {}


def _ensure_axon_hooks_shim():
    """bass_utils imports antenv.axon_hooks when BASS_TRACE is set; provide a
    null shim so tracing degrades gracefully if the module is absent."""
    try:
        import antenv.axon_hooks  # noqa: F401
        return
    except ImportError:
        pass
    try:
        import antenv  # noqa: F401
    except ImportError:
        return
    mod = types.ModuleType("antenv.axon_hooks")
    _state = {"hook": None}
    mod.set_axon_ntff_profile_hook = lambda h: _state.__setitem__("hook", h)
    mod.get_axon_ntff_profile_hook = lambda: _state["hook"]
    sys.modules["antenv.axon_hooks"] = mod


def _build():
    from concourse import bacc, mybir, tile

    f32 = mybir.dt.float32
    bf16 = mybir.dt.bfloat16
    FT = mybir.ActivationFunctionType
    AL = mybir.AluOpType

    nc = bacc.Bacc("TRN2", target_bir_lowering=False, debug=False,
                   num_devices=N_CORES)

    XTd = nc.declare_dram_parameter("XT", [2, 128, BS], bf16, isOutput=False)
    Pd = nc.declare_dram_parameter("P", [RANK, H, QK], bf16, isOutput=False)
    W1d = nc.declare_dram_parameter("W1", [2, 128, H], bf16, isOutput=False)
    W2d = nc.declare_dram_parameter("W2", [2, 128, OUT], bf16, isOutput=False)
    W3d = nc.declare_dram_parameter("W3", [H, OUT], bf16, isOutput=False)
    BCd = nc.declare_dram_parameter("bcol", [128, 4], f32, isOutput=False)
    BNd = nc.declare_dram_parameter("bn", [H, 4], f32, isOutput=False)
    ONEd = nc.declare_dram_parameter("ones", [1, 128], bf16, isOutput=False)
    OUTd = nc.declare_dram_parameter("out", [OUT, BS], f32, isOutput=True)

    rg = [list(range(N_CORES))]
    deferred_waits = []  # (inst, sem, val): applied post-scheduling; the
    # single-core scheduling sim cannot model cross-core sem increments.

    with tile.TileContext(nc) as tc:
        rsem = nc.alloc_semaphore("rsem")
        lsem = nc.alloc_semaphore("lsem")
        ssem = nc.alloc_semaphore("ssem")
        scum = [0]  # cumulative stats-op count gating each round's trigger
        with (
            tc.tile_pool(name="const", bufs=1) as cpool,
            tc.tile_pool(name="wtab", bufs=1) as wpool,
            tc.tile_pool(name="ppool", bufs=2) as ppool,
            tc.tile_pool(name="mpool", bufs=3) as mpool,
            tc.tile_pool(name="zit", bufs=2) as zitpool,
            tc.tile_pool(name="small", bufs=4) as spool,
            tc.tile_pool(name="gpool", bufs=2) as gpool,
            tc.tile_pool(name="psmain", bufs=2, space="PSUM") as psmain,
            tc.tile_pool(name="pswb", bufs=2, space="PSUM") as pswb,
            tc.tile_pool(name="psaux", bufs=2, space="PSUM") as psaux,
            tc.tile_pool(name="dram", bufs=4, space="DRAM") as dpool,
        ):
            # ---------------- constants ----------------
            xt = cpool.tile([128, 2 * BS], bf16, tag="xt")
            for c in range(2):
                nc.sync.dma_start(xt[:, c * BS:(c + 1) * BS], XTd[c])
            w1 = cpool.tile([128, 2 * H], bf16, tag="w1")
            w2 = cpool.tile([128, 2 * OUT], bf16, tag="w2")
            for c in range(2):
                nc.sync.dma_start(w1[:, c * H:(c + 1) * H], W1d[c])
                nc.sync.dma_start(w2[:, c * OUT:(c + 1) * OUT], W2d[c])
            w3 = cpool.tile([H, OUT], bf16, tag="w3")
            nc.sync.dma_start(w3[:], W3d[:])
            bcol = cpool.tile([128, 4], f32, tag="bcol")
            nc.sync.dma_start(bcol[:], BCd[:])
            bn = cpool.tile([H, 4], f32, tag="bn")
            nc.sync.dma_start(bn[:], BNd[:])
            ones = cpool.tile([1, 128], bf16, tag="ones")
            nc.sync.dma_start(ones[:], ONEd[:])
            epsc = cpool.tile([H, 1], f32, tag="epsc")
            nc.vector.memset(epsc[:], EPS)
            yt = cpool.tile([H, BS], f32, tag="yt")
            nc.vector.memset(yt[:], 0.0)

            # Cross-core stats-exchange buffers (stable addresses, shared by
            # the SPMD program across cores; parity-alternated to tolerate a
            # one-round skew between cores).
            gA = cpool.tile([H, 16], f32, tag="gA")
            gB = cpool.tile([H, 16], f32, tag="gB")
            g8 = cpool.tile([H, 64], f32, tag="g8")
            nc.gpsimd.sem_clear(rsem)
            nc.gpsimd.sem_clear(lsem)
            nc.gpsimd.sem_clear(ssem)

            # Early dummy collective: absorbs cross-core launch skew and
            # guarantees every core cleared its sems before any remote sends.
            dsrc = dpool.tile([H, 2], f32, tag="ccsrc2")
            ddst = dpool.tile([N_CORES * H, 2], f32, tag="ccdst2")
            nc.sync.dma_start(dsrc[:], bn[:, 0:2])
            nc.gpsimd.collective_compute(
                "AllGather", AL.bypass, replica_groups=rg,
                ins=[dsrc.opt()], outs=[ddst.opt()],
            )
            tc.no_sync_barrier()
            # Early Sqrt: preload the ACT table set before the first sync.
            sqd = spool.tile([H, 1], f32, tag="sqd")
            nc.scalar.activation(sqd[:], epsc[:], FT.Sqrt)

            # ---------------- Z^T = relu(W1^T @ X^T + b1) ----------------
            psz = psaux.tile([128, BS], f32, tag="aux")
            for c in range(2):
                nc.tensor.matmul(
                    psz[:], lhsT=w1[:, c * H:(c + 1) * H],
                    rhs=xt[:, c * BS:(c + 1) * BS],
                    start=(c == 0), stop=(c == 1))
            zt = cpool.tile([128, BS], bf16, tag="zt")
            nc.scalar.activation(zt[:], psz[:], FT.Relu, bias=bcol[:, 0:1])

            # r2t = relu(W2^T @ X^T + b2)  (independent of the scan)
            psr2 = psaux.tile([128, BS], f32, tag="aux")
            for c in range(2):
                nc.tensor.matmul(
                    psr2[:], lhsT=w2[:, c * OUT:(c + 1) * OUT],
                    rhs=xt[:, c * BS:(c + 1) * BS],
                    start=(c == 0), stop=(c == 1))
            r2t = cpool.tile([128, BS], bf16, tag="r2t")
            nc.scalar.activation(r2t[:], psr2[:], FT.Relu, bias=bcol[:, 1:2])

            # ------- W[p, (q, b)] = Z^T[q, b]: partition-broadcast table ----
            # Flatten Z^T into one partition (DRAM bounce), then ones-matmul
            # broadcasts it across 128 partitions; evac split ACT/DVE.
            zfd = dpool.tile([1, H * BS], bf16, tag="zflat")
            nc.scalar.dma_start(
                zfd[:].rearrange("o (q b) -> (o q) b", q=H), zt[:])
            wtab = wpool.tile([128, H * BS], bf16, tag="wtab")
            NWB = 1024  # f32 psum group: 2 banks, 2 matmuls of N=512
            for j in range(H * BS // NWB):
                zflat = mpool.tile([1, NWB], bf16, tag="zflat_sb")
                nc.scalar.dma_start(zflat[:],
                                    zfd[0:1, j * NWB:(j + 1) * NWB])
                pw = pswb.tile([128, NWB], f32, tag="wb")
                for i in range(2):
                    nc.tensor.matmul(
                        pw[:, i * 512:(i + 1) * 512], lhsT=ones[:],
                        rhs=zflat[0:1, i * 512:(i + 1) * 512],
                        start=True, stop=True)
                dst = wtab[:, j * NWB:(j + 1) * NWB]
                if j % 2 == 0:
                    nc.vector.tensor_copy(dst, pw[:])
                else:
                    nc.scalar.activation(dst, pw[:], FT.Copy)

            # ---------------- scan over ranks ----------------
            zit = zt
            for r in range(RANK):
                p_sb = ppool.tile([128, QK], bf16, tag="p")
                for pc in range(PCH):
                    w = QK // PCH
                    nc.sync.dma_start(p_sb[:, pc * w:(pc + 1) * w],
                                      Pd[r][:, pc * w:(pc + 1) * w])

                # stats tile + remote-broadcast descriptor prep (data is read
                # only at trigger time, after the stats are written)
                last = (r == RANK - 1)
                stw = 8 if last else 2
                stl = spool.tile([H, stw], f32, tag=f"stl{stw}")
                gt = g8 if last else (gA, gB)[r % 2]
                for k in range(N_CORES):
                    rd = [None] * N_CORES
                    rd[k] = (0, k)
                    nc.gpsimd.remote_dma_broadcast(
                        gt[:, k * stw:(k + 1) * stw], stl[:],
                        remote_sem=rsem, local_sem=lsem, rdests=rd)

                ps = psmain.tile([128, BS], f32, tag="mm")
                for c in range(NCH):
                    m = mpool.tile([128, NQC * BS], bf16, tag="m")
                    nc.vector.tensor_tensor(
                        m[:].rearrange("p (a b) -> p a b", b=BS),
                        wtab[:, c * NQC * BS:(c + 1) * NQC * BS].rearrange(
                            "p (a b) -> p a b", b=BS),
                        zit[:].rearrange("p (o b) -> p o b", o=1
                                         ).broadcast_to((128, NQC, BS)),
                        AL.mult)
                    for i in range(NQC):
                        q = c * NQC + i
                        nc.tensor.matmul(
                            ps[:], lhsT=p_sb[:, q * H:(q + 1) * H],
                            rhs=m[:, i * BS:(i + 1) * BS],
                            start=(q == 0), stop=(q == H - 1))

                # stats + evac:  ghat = G^T (bf16), stl = [S1, S2, ...]
                # (each stl-writing ACT op bumps ssem: gates the send trigger)
                ghat = gpool.tile([H, BS], bf16, tag="ghat")
                scr = gpool.tile([H, BS], bf16, tag="scr")
                nc.scalar.activation(ghat[:], ps[:], FT.Copy,
                                     accum_out=stl[:, 0:1])
                nc.scalar.activation(scr[:], ps[:], FT.Square,
                                     accum_out=stl[:, 1:2])
                if last:
                    # Piggyback Y-BN inputs on the final sync: with
                    # R = sum_{r<7} Zi (= yt now) and Zi_8 = a*G + c,
                    # SumY and SumY^2 expand in closed form from
                    # [S1G, S2G, S1R, S2R, SX] -- no 9th sync.
                    nc.scalar.activation(scr[:], yt[:], FT.Copy,
                                         accum_out=stl[:, 2:3])
                    nc.scalar.activation(scr[:], yt[:], FT.Square,
                                         accum_out=stl[:, 3:4])
                    scry2 = gpool.tile([H, BS], bf16, tag="scry2")
                    nc.vector.tensor_tensor(scry2[:], yt[:], ghat[:], AL.mult)
                    nc.scalar.activation(scr[:], scry2[:], FT.Copy,
                                         accum_out=stl[:, 4:5])
                # stats-complete marker: the copy reads every stl column, so
                # it orders after all stats writers (cross-engine via Tile);
                # the pinned trailing NOP bumps ssem to release the trigger
                # (DVE FIFO: the NOP retires only after the copy completes).
                scopy = spool.tile([H, stw], f32, tag=f"stlc{stw}")
                nc.vector.tensor_copy(scopy[:], stl[:])
                tc.no_sync_barrier()
                nc.vector.engine_nop().then_inc(ssem, 1)
                scum[0] += 1

                # ---- cross-core exchange of stats (remote SBUF DMA) ----
                a_ap, c_ap, stg = _bn_sync(nc, tc, spool, stl, gt, r,
                                           rsem, ssem, scum[0],
                                           deferred_waits, bn,
                                           gcol=0, bcol=1, epsc=epsc)

                # Zi+1^T = a*G^T + c  (ACT per-partition scale+bias)
                zit_next = zitpool.tile([H, BS], bf16, tag="zit")
                nc.scalar.activation(zit_next[:], ghat[:], FT.Identity,
                                     scale=a_ap, bias=c_ap)
                nc.vector.tensor_tensor(yt[:], yt[:], zit_next[:], AL.add)
                zit = zit_next

            # ------- Y BN from closed-form global sums (no extra sync) ----
            # stg (global): [S1G, S2G, S1R, S2R, SX]; a_ap/c_ap = rank-7 BN.
            # SumY  = (S1R + a*S1G + B*c) / 8
            # SumY2 = (S2R + 2*(a*SX + c*S1R)
            #          + a^2*S2G + 2*a*c*S1G + B*c^2) / 64
            S1G, S2G = stg[:, 0:1], stg[:, 1:2]
            S1R, S2R = stg[:, 2:3], stg[:, 3:4]
            SX = stg[:, 4:5]
            w = spool.tile([H, 10], f32, tag="ywork")
            nc.vector.tensor_tensor(w[:, 0:1], a_ap, S1G, AL.mult)   # a*S1G
            nc.vector.tensor_scalar(w[:, 1:2], c_ap, float(B), w[:, 0:1],
                                    AL.mult, AL.add)                 # S1Z
            nc.vector.tensor_tensor(w[:, 2:3], w[:, 1:2], S1R, AL.add)  # SumY*8
            nc.vector.tensor_tensor(w[:, 3:4], a_ap, SX, AL.mult)
            nc.vector.tensor_tensor(w[:, 4:5], c_ap, S1R, AL.mult)
            nc.vector.tensor_tensor(w[:, 3:4], w[:, 3:4], w[:, 4:5], AL.add)
            # w3 = SRZ = a*SX + c*S1R
            nc.vector.tensor_tensor(w[:, 5:6], a_ap, a_ap, AL.mult)  # a^2
            nc.vector.tensor_tensor(w[:, 5:6], w[:, 5:6], S2G, AL.mult)
            nc.vector.tensor_tensor(w[:, 6:7], a_ap, c_ap, AL.mult)  # a*c
            nc.vector.tensor_tensor(w[:, 6:7], w[:, 6:7], S1G, AL.mult)
            nc.vector.tensor_tensor(w[:, 7:8], c_ap, c_ap, AL.mult)  # c^2
            nc.vector.tensor_scalar(w[:, 7:8], w[:, 7:8], float(B), None,
                                    AL.mult)
            # S2Z = a^2*S2G + 2*a*c*S1G + B*c^2
            nc.vector.tensor_scalar(w[:, 6:7], w[:, 6:7], 2.0, None, AL.mult)
            nc.vector.tensor_tensor(w[:, 5:6], w[:, 5:6], w[:, 6:7], AL.add)
            nc.vector.tensor_tensor(w[:, 5:6], w[:, 5:6], w[:, 7:8], AL.add)
            nc.vector.tensor_scalar(w[:, 3:4], w[:, 3:4], 2.0, None, AL.mult)
            nc.vector.tensor_tensor(w[:, 8:9], S2R, w[:, 3:4], AL.add)
            nc.vector.tensor_tensor(w[:, 8:9], w[:, 8:9], w[:, 5:6], AL.add)
            # w8 = SumY2*64;  mean/var of Y:
            nc.vector.tensor_scalar(w[:, 2:3], w[:, 2:3], 1.0 / (8.0 * B),
                                    None, AL.mult)                   # mY
            nc.vector.tensor_scalar(w[:, 8:9], w[:, 8:9], 1.0 / (64.0 * B),
                                    None, AL.mult)                   # E[Y^2]
            nc.vector.tensor_tensor(w[:, 9:10], w[:, 2:3], w[:, 2:3], AL.mult)
            nc.vector.tensor_scalar(w[:, 9:10], w[:, 9:10], -1.0, w[:, 8:9],
                                    AL.mult, AL.add)                 # var
            sdy = spool.tile([H, 4], f32, tag="ycoef")
            nc.scalar.activation(sdy[:, 0:1], w[:, 9:10], FT.Sqrt,
                                 bias=epsc[:])
            nc.vector.reciprocal(sdy[:, 1:2], sdy[:, 0:1])
            nc.vector.tensor_tensor(sdy[:, 1:2], sdy[:, 1:2], bn[:, 2:3],
                                    AL.mult)                         # ay
            nc.vector.tensor_tensor(sdy[:, 2:3], w[:, 2:3], sdy[:, 1:2],
                                    AL.mult)
            nc.vector.tensor_tensor(sdy[:, 2:3], bn[:, 3:4], sdy[:, 2:3],
                                    AL.subtract)                     # cy
            nc.vector.tensor_scalar(sdy[:, 3:4], sdy[:, 1:2], 0.125, None,
                                    AL.mult)                         # ay/8
            ybn = spool.tile([H, BS], bf16, tag="ybn")
            nc.vector.tensor_scalar(ybn[:], yt[:], sdy[:, 3:4], sdy[:, 2:3],
                                    AL.mult, AL.add)

            # -------- out^T = relu(relu(W3^T@Ybn^T+b3) + r2t) --------
            psA = psaux.tile([128, BS], f32, tag="aux")
            nc.tensor.matmul(psA[:], lhsT=w3[:], rhs=ybn[:],
                             start=True, stop=True)
            r1t = spool.tile([128, BS], f32, tag="r1t")
            nc.scalar.activation(r1t[:], psA[:], FT.Relu, bias=bcol[:, 2:3])
            s = spool.tile([128, BS], f32, tag="s")
            nc.vector.tensor_tensor(s[:], r1t[:], r2t[:], AL.add)
            of = spool.tile([128, BS], f32, tag="of")
            nc.scalar.activation(of[:], s[:], FT.Relu)
            nc.sync.dma_start(OUTd[:], of[:])

    for inst, sem, val in deferred_waits:
        inst.wait_op(sem, val, "sem-ge")

    nc.compile()
    return nc


def _bn_sync(nc, tc, spool, stl, gath, rnd, rsem, ssem, scum, deferred, bn,
             gcol, bcol, epsc):
    """Exchange per-core (H, W) stats [sum, sumsq, ...] via remote SBUF DMA
    broadcasts (descriptors prepped at rank top), reduce across the 8 cores,
    compute affine coeffs a, c s.t. BN(x) = a*x + c (per-partition).
    Returns (a, c, global_sums)."""
    from concourse import mybir

    f32 = mybir.dt.float32
    FT = mybir.ActivationFunctionType
    AL = mybir.AluOpType

    W = stl.shape[1]
    tc.no_sync_barrier()  # all 8 preps must precede the trigger
    trig = nc.gpsimd.trigger_dma(count=None)
    # gate the SDMA reads of stl on the stats being written (cross-engine
    # runtime dep; Tile's prep->trigger edge is scheduler-order only)
    deferred.append((trig, ssem, scum))
    tc.no_sync_barrier()
    wi = nc.vector.wait_ge(rsem, 0)
    deferred.append((wi, rsem, 16 * (rnd + 1)))
    tc.no_sync_barrier()
    # reduce over cores: slot-major layout (slot k holds some peer's stats —
    # the slot->core map is a permutation, irrelevant for a sum)
    r4 = spool.tile([H, 4 * W], f32, tag=f"r4{W}")
    nc.vector.tensor_tensor(r4[:], gath[:, 0:4 * W], gath[:, 4 * W:8 * W],
                            AL.add)
    r2 = spool.tile([H, 2 * W], f32, tag=f"r2s{W}")
    nc.vector.tensor_tensor(r2[:], r4[:, 0:2 * W], r4[:, 2 * W:4 * W], AL.add)
    st = spool.tile([H, W], f32, tag=f"stg{W}")
    nc.vector.tensor_tensor(st[:], r2[:, 0:W], r2[:, W:2 * W], AL.add)

    cf = spool.tile([H, 8], f32, tag="cf")
    me2 = cf[:, 0:2]   # [mean, E[x^2]]
    m = cf[:, 0:1]
    ex2 = cf[:, 1:2]
    v = cf[:, 2:3]
    sd = cf[:, 3:4]
    rinv = cf[:, 4:5]
    a = cf[:, 5:6]
    t = cf[:, 6:7]
    c = cf[:, 7:8]
    nc.vector.tensor_scalar(me2, st[:, 0:2], 1.0 / B, None, AL.mult)
    msq = spool.tile([H, 1], f32, tag="msq")
    nc.vector.tensor_tensor(msq[:], m, m, AL.mult)
    nc.vector.tensor_scalar(v, msq[:], -1.0, ex2, AL.mult, AL.add)
    nc.scalar.activation(sd, v, FT.Sqrt, bias=epsc[:])
    nc.vector.reciprocal(rinv, sd)
    nc.vector.tensor_tensor(a, rinv, bn[:, gcol:gcol + 1], AL.mult)
    nc.vector.tensor_tensor(t, m, a, AL.mult)
    nc.vector.tensor_tensor(c, bn[:, bcol:bcol + 1], t, AL.subtract)
    return a, c, st


def _prep_inputs(X, W1, b1, W2, b2, W3, b3, P, gz, bz, gy, by):
    bf = ml_dtypes.bfloat16
    per_core = []
    P_b = np.ascontiguousarray(P.reshape(RANK, H, QK)).astype(bf)
    W1_b = np.ascontiguousarray(W1.reshape(2, 128, H)).astype(bf)
    W2_b = np.ascontiguousarray(W2.reshape(2, 128, OUT)).astype(bf)
    W3_b = np.ascontiguousarray(W3).astype(bf)
    bc = np.zeros((128, 4), np.float32)
    bc[:, 0] = b1
    bc[:, 1] = b2
    bc[:, 2] = b3
    bnc = np.stack([gz, bz, gy, by], axis=1).astype(np.float32)
    ones = np.ones((1, 128), np.float32).astype(bf)
    for s in range(N_CORES):
        Xs = X[s * BS:(s + 1) * BS]
        XT = np.ascontiguousarray(Xs.T.reshape(2, 128, BS)).astype(bf)
        per_core.append({
            "XT": XT, "P": P_b, "W1": W1_b, "W2": W2_b, "W3": W3_b,
            "bcol": bc, "bn": bnc, "ones": ones,
        })
    return per_core


def kernel(**inputs):
    _ensure_axon_hooks_shim()
    from concourse.bass_utils import run_bass_kernel_spmd

    if "nc" not in _cache:
        _cache["nc"] = _build()
    nc = _cache["nc"]

    in_maps = _prep_inputs(**{k: np.asarray(v) for k, v in inputs.items()})
    res = run_bass_kernel_spmd(nc, in_maps, core_ids=list(range(N_CORES)))
    out = np.concatenate([m["out"].T for m in res.results], axis=0)
    return np.ascontiguousarray(out).astype(np.float32)


if __name__ == "__main__":
    import reference as R

    inputs = {k: np.asarray(v) for k, v in R.setup_inputs().items()}
    got = kernel(**inputs)
    exp = np.asarray(R.reference(**R.setup_inputs()))
    rel = np.linalg.norm(got - exp) / np.linalg.norm(exp)
    print("rel l2:", rel)


# revision 25
# speedup vs baseline: 1.2592x; 1.2592x over previous
"""Distributed Trainium2 kernel for nn_ApaBlock (8 NeuronCores, data-parallel).

Transposed dataflow (per core, batch shard of 256 rows), no PE transposes:
  Z^T = relu(W1^T @ X^T + b1)              [h, b] via lhsT=W1 (natural)
  W[p,(q,b)] = Z^T[q,b]                    partition-broadcast table, built
                                           once via ones-matmuls + evac
  scan over 8 ranks:
    M_q[p,b] = Zi^T[p,b] * Z^T[q,b]        DVE tensor_tensor bf16 SBUF (2x)
    G^T[k,b] += P_q^T @ M_q                128 accumulating matmuls (N=256)
    stats via ACT accum_out during PSUM evac; 1KB AllGather (sync-BN);
    Zi+1^T = a*G^T + c on ACT             (per-partition scale+bias)
  Y-BN via closed-form global sums piggybacked on the rank-7 sync.
  out^T = relu(relu(W3^T@Ybn^T+b3) + relu(W2^T@X^T+b2)); host transposes.

vs baseline: TensorE streaming work halved (no identity-matmul q-reduction),
DVE scale moved from 1x PSUM-sourced to 2x SBUF bf16, zero PE transposes.
"""

import os
import sys
import types

if "/opt/trn_rl_repo" not in sys.path:
    sys.path.insert(0, "/opt/trn_rl_repo")

import numpy as np
import ml_dtypes

N_CORES = 8
B, IN, H, OUT, RANK = 2048, 256, 128, 128, 8
BS = B // N_CORES  # 256 rows per core
EPS = 1e-5
QK = H * H  # 16384
NQC = 16            # max q values per M chunk
MCH = [4, 12] + [16] * 7   # chunk sizes (small first chunk: PE lead-in)
PCH = 4             # P dma split per rank

_cache = {}


def _ensure_axon_hooks_shim():
    """bass_utils imports antenv.axon_hooks when BASS_TRACE is set; provide a
    null shim so tracing degrades gracefully if the module is absent."""
    try:
        import antenv.axon_hooks  # noqa: F401
        return
    except ImportError:
        pass
    try:
        import antenv  # noqa: F401
    except ImportError:
        return
    mod = types.ModuleType("antenv.axon_hooks")
    _state = {"hook": None}
    mod.set_axon_ntff_profile_hook = lambda h: _state.__setitem__("hook", h)
    mod.get_axon_ntff_profile_hook = lambda: _state["hook"]
    sys.modules["antenv.axon_hooks"] = mod


def _build():
    from concourse import bacc, mybir, tile

    f32 = mybir.dt.float32
    bf16 = mybir.dt.bfloat16
    FT = mybir.ActivationFunctionType
    AL = mybir.AluOpType

    nc = bacc.Bacc("TRN2", target_bir_lowering=False, debug=False,
                   num_devices=N_CORES)

    XTd = nc.declare_dram_parameter("XT", [2, 128, BS], bf16, isOutput=False)
    Pd = nc.declare_dram_parameter("P", [RANK, H, QK], bf16, isOutput=False)
    W1d = nc.declare_dram_parameter("W1", [2, 128, H], bf16, isOutput=False)
    W2d = nc.declare_dram_parameter("W2", [2, 128, OUT], bf16, isOutput=False)
    W3d = nc.declare_dram_parameter("W3", [H, OUT], bf16, isOutput=False)
    BCd = nc.declare_dram_parameter("bcol", [128, 4], f32, isOutput=False)
    BNd = nc.declare_dram_parameter("bn", [H, 4], f32, isOutput=False)
    ONEd = nc.declare_dram_parameter("ones", [1, 128], bf16, isOutput=False)
    OUTd = nc.declare_dram_parameter("out", [OUT, BS], f32, isOutput=True)

    rg = [list(range(N_CORES))]
    deferred_waits = []  # (inst, sem, val): applied post-scheduling; the
    # single-core scheduling sim cannot model cross-core sem increments.

    with tile.TileContext(nc) as tc:
        with (
            tc.tile_pool(name="const", bufs=1) as cpool,
            tc.tile_pool(name="wtab", bufs=1) as wpool,
            tc.tile_pool(name="ppool", bufs=3) as ppool,
            tc.tile_pool(name="mpool", bufs=2) as mpool,
            tc.tile_pool(name="zit", bufs=2) as zitpool,
            tc.tile_pool(name="small", bufs=4) as spool,
            tc.tile_pool(name="gpool", bufs=2) as gpool,
            tc.tile_pool(name="psmain", bufs=2, space="PSUM") as psmain,
            tc.tile_pool(name="pswb", bufs=2, space="PSUM") as pswb,
            tc.tile_pool(name="psaux", bufs=2, space="PSUM") as psaux,
            tc.tile_pool(name="dram", bufs=4, space="DRAM") as dpool,
        ):
            # ---------------- constants ----------------
            xt = cpool.tile([128, 2 * BS], bf16, tag="xt")
            for c in range(2):
                nc.sync.dma_start(xt[:, c * BS:(c + 1) * BS], XTd[c])
            w1 = cpool.tile([128, 2 * H], bf16, tag="w1")
            w2 = cpool.tile([128, 2 * OUT], bf16, tag="w2")
            for c in range(2):
                nc.sync.dma_start(w1[:, c * H:(c + 1) * H], W1d[c])
                nc.sync.dma_start(w2[:, c * OUT:(c + 1) * OUT], W2d[c])
            w3 = cpool.tile([H, OUT], bf16, tag="w3")
            nc.sync.dma_start(w3[:], W3d[:])
            bcol = cpool.tile([128, 4], f32, tag="bcol")
            nc.sync.dma_start(bcol[:], BCd[:])
            bn = cpool.tile([H, 4], f32, tag="bn")
            nc.sync.dma_start(bn[:], BNd[:])
            ones = cpool.tile([1, 128], bf16, tag="ones")
            nc.sync.dma_start(ones[:], ONEd[:])
            epsc = cpool.tile([H, 1], f32, tag="epsc")
            nc.vector.memset(epsc[:], EPS)
            yt = cpool.tile([H, BS], f32, tag="yt")
            nc.vector.memset(yt[:], 0.0)

            # Early dummy collective: absorbs cross-core launch skew.
            dsrc = dpool.tile([H, 2], f32, tag="ccsrc2")
            ddst = dpool.tile([N_CORES * H, 2], f32, tag="ccdst2")
            nc.sync.dma_start(dsrc[:], bn[:, 0:2])
            nc.gpsimd.collective_compute(
                "AllGather", AL.bypass, replica_groups=rg,
                ins=[dsrc.opt()], outs=[ddst.opt()],
            )
            # Early Sqrt: preload the ACT table set before the first sync.
            sqd = spool.tile([H, 1], f32, tag="sqd")
            nc.scalar.activation(sqd[:], epsc[:], FT.Sqrt)

            # ---------------- Z^T = relu(W1^T @ X^T + b1) ----------------
            psz = psaux.tile([128, BS], f32, tag="aux")
            for c in range(2):
                nc.tensor.matmul(
                    psz[:], lhsT=w1[:, c * H:(c + 1) * H],
                    rhs=xt[:, c * BS:(c + 1) * BS],
                    start=(c == 0), stop=(c == 1))
            zt = cpool.tile([128, BS], bf16, tag="zt")
            nc.scalar.activation(zt[:], psz[:], FT.Relu, bias=bcol[:, 0:1])

            # r2t = relu(W2^T @ X^T + b2)  (independent of the scan)
            psr2 = psaux.tile([128, BS], f32, tag="aux")
            for c in range(2):
                nc.tensor.matmul(
                    psr2[:], lhsT=w2[:, c * OUT:(c + 1) * OUT],
                    rhs=xt[:, c * BS:(c + 1) * BS],
                    start=(c == 0), stop=(c == 1))
            r2t = cpool.tile([128, BS], bf16, tag="r2t")
            nc.scalar.activation(r2t[:], psr2[:], FT.Relu, bias=bcol[:, 1:2])

            # ------- W[p, (q, b)] = Z^T[q, b]: partition-broadcast table ----
            # Flatten Z^T into one partition (DRAM bounce), then ones-matmul
            # broadcasts it across 128 partitions; evac split ACT/DVE.
            zfd = dpool.tile([1, H * BS], bf16, tag="zflat")
            nc.scalar.dma_start(
                zfd[:].rearrange("o (q b) -> (o q) b", q=H), zt[:])
            wtab = wpool.tile([128, H * BS], bf16, tag="wtab")
            NWB = 1024  # f32 psum group: 2 banks, 2 matmuls of N=512
            for j in range(H * BS // NWB):
                zflat = mpool.tile([1, NWB], bf16, tag="zflat_sb")
                nc.scalar.dma_start(zflat[:],
                                    zfd[0:1, j * NWB:(j + 1) * NWB])
                pw = pswb.tile([128, NWB], f32, tag="wb")
                for i in range(2):
                    nc.tensor.matmul(
                        pw[:, i * 512:(i + 1) * 512], lhsT=ones[:],
                        rhs=zflat[0:1, i * 512:(i + 1) * 512],
                        start=True, stop=True)
                dst = wtab[:, j * NWB:(j + 1) * NWB]
                if j % 2 == 0:
                    nc.vector.tensor_copy(dst, pw[:])
                else:
                    nc.scalar.activation(dst, pw[:], FT.Copy)

            # ---------------- scan over ranks ----------------
            zit = zt
            for r in range(RANK):
                p_sb = ppool.tile([128, QK], bf16, tag="p")
                for pc in range(PCH):
                    w = QK // PCH
                    nc.sync.dma_start(p_sb[:, pc * w:(pc + 1) * w],
                                      Pd[r][:, pc * w:(pc + 1) * w])

                last = (r == RANK - 1)
                stw = 8 if last else 2
                stl = spool.tile([H, stw], f32, tag=f"stl{stw}")

                ps = psmain.tile([128, BS], f32, tag="mm")
                q0 = 0
                for nq in MCH:
                    m = mpool.tile([128, NQC * BS], bf16, tag="m")
                    nc.vector.tensor_tensor(
                        m[:, 0:nq * BS].rearrange("p (a b) -> p a b", b=BS),
                        wtab[:, q0 * BS:(q0 + nq) * BS].rearrange(
                            "p (a b) -> p a b", b=BS),
                        zit[:].rearrange("p (o b) -> p o b", o=1
                                         ).broadcast_to((128, nq, BS)),
                        AL.mult)
                    for i in range(nq):
                        q = q0 + i
                        nc.tensor.matmul(
                            ps[:], lhsT=p_sb[:, q * H:(q + 1) * H],
                            rhs=m[:, i * BS:(i + 1) * BS],
                            start=(q == 0), stop=(q == H - 1))
                    q0 += nq

                # stats + evac:  ghat = G^T (bf16), stl = [S1, S2, ...]
                ghat = gpool.tile([H, BS], bf16, tag="ghat")
                scr = gpool.tile([H, BS], bf16, tag="scr")
                nc.scalar.activation(ghat[:], ps[:], FT.Copy,
                                     accum_out=stl[:, 0:1])
                nc.scalar.activation(scr[:], ps[:], FT.Square,
                                     accum_out=stl[:, 1:2])
                if last:
                    # Piggyback Y-BN inputs on the final sync: with
                    # R = sum_{r<7} Zi (= yt now) and Zi_8 = a*G + c,
                    # SumY and SumY^2 expand in closed form from
                    # [S1G, S2G, S1R, S2R, SX] -- no 9th sync.
                    nc.scalar.activation(scr[:], yt[:], FT.Copy,
                                         accum_out=stl[:, 2:3])
                    nc.scalar.activation(scr[:], yt[:], FT.Square,
                                         accum_out=stl[:, 3:4])
                    scry2 = gpool.tile([H, BS], bf16, tag="scry2")
                    nc.vector.tensor_tensor(scry2[:], yt[:], ghat[:], AL.mult)
                    nc.scalar.activation(scr[:], scry2[:], FT.Copy,
                                         accum_out=stl[:, 4:5])

                # ---- cross-core AllGather of stats ----
                a_ap, c_ap, stg = _bn_sync(nc, dpool, spool, stl, bn,
                                           gcol=0, bcol=1, epsc=epsc)

                # Zi+1^T = a*G^T + c  (ACT per-partition scale+bias)
                zit_next = zitpool.tile([H, BS], bf16, tag="zit")
                nc.scalar.activation(zit_next[:], ghat[:], FT.Identity,
                                     scale=a_ap, bias=c_ap)
                nc.vector.tensor_tensor(yt[:], yt[:], zit_next[:], AL.add)
                zit = zit_next

            # ------- Y BN from closed-form global sums (no extra sync) ----
            # stg (global): [S1G, S2G, S1R, S2R, SX]; a_ap/c_ap = rank-7 BN.
            # SumY  = (S1R + a*S1G + B*c) / 8
            # SumY2 = (S2R + 2*(a*SX + c*S1R)
            #          + a^2*S2G + 2*a*c*S1G + B*c^2) / 64
            S1G, S2G = stg[:, 0:1], stg[:, 1:2]
            S1R, S2R = stg[:, 2:3], stg[:, 3:4]
            SX = stg[:, 4:5]
            w = spool.tile([H, 10], f32, tag="ywork")
            nc.vector.tensor_tensor(w[:, 0:1], a_ap, S1G, AL.mult)   # a*S1G
            nc.vector.tensor_scalar(w[:, 1:2], c_ap, float(B), w[:, 0:1],
                                    AL.mult, AL.add)                 # S1Z
            nc.vector.tensor_tensor(w[:, 2:3], w[:, 1:2], S1R, AL.add)  # SumY*8
            nc.vector.tensor_tensor(w[:, 3:4], a_ap, SX, AL.mult)
            nc.vector.tensor_tensor(w[:, 4:5], c_ap, S1R, AL.mult)
            nc.vector.tensor_tensor(w[:, 3:4], w[:, 3:4], w[:, 4:5], AL.add)
            # w3 = SRZ = a*SX + c*S1R
            nc.vector.tensor_tensor(w[:, 5:6], a_ap, a_ap, AL.mult)  # a^2
            nc.vector.tensor_tensor(w[:, 5:6], w[:, 5:6], S2G, AL.mult)
            nc.vector.tensor_tensor(w[:, 6:7], a_ap, c_ap, AL.mult)  # a*c
            nc.vector.tensor_tensor(w[:, 6:7], w[:, 6:7], S1G, AL.mult)
            nc.vector.tensor_tensor(w[:, 7:8], c_ap, c_ap, AL.mult)  # c^2
            nc.vector.tensor_scalar(w[:, 7:8], w[:, 7:8], float(B), None,
                                    AL.mult)
            # S2Z = a^2*S2G + 2*a*c*S1G + B*c^2
            nc.vector.tensor_scalar(w[:, 6:7], w[:, 6:7], 2.0, None, AL.mult)
            nc.vector.tensor_tensor(w[:, 5:6], w[:, 5:6], w[:, 6:7], AL.add)
            nc.vector.tensor_tensor(w[:, 5:6], w[:, 5:6], w[:, 7:8], AL.add)
            nc.vector.tensor_scalar(w[:, 3:4], w[:, 3:4], 2.0, None, AL.mult)
            nc.vector.tensor_tensor(w[:, 8:9], S2R, w[:, 3:4], AL.add)
            nc.vector.tensor_tensor(w[:, 8:9], w[:, 8:9], w[:, 5:6], AL.add)
            # w8 = SumY2*64;  mean/var of Y:
            nc.vector.tensor_scalar(w[:, 2:3], w[:, 2:3], 1.0 / (8.0 * B),
                                    None, AL.mult)                   # mY
            nc.vector.tensor_scalar(w[:, 8:9], w[:, 8:9], 1.0 / (64.0 * B),
                                    None, AL.mult)                   # E[Y^2]
            nc.vector.tensor_tensor(w[:, 9:10], w[:, 2:3], w[:, 2:3], AL.mult)
            nc.vector.tensor_scalar(w[:, 9:10], w[:, 9:10], -1.0, w[:, 8:9],
                                    AL.mult, AL.add)                 # var
            sdy = spool.tile([H, 4], f32, tag="ycoef")
            nc.scalar.activation(sdy[:, 0:1], w[:, 9:10], FT.Sqrt,
                                 bias=epsc[:])
            nc.vector.reciprocal(sdy[:, 1:2], sdy[:, 0:1])
            nc.vector.tensor_tensor(sdy[:, 1:2], sdy[:, 1:2], bn[:, 2:3],
                                    AL.mult)                         # ay
            nc.vector.tensor_tensor(sdy[:, 2:3], w[:, 2:3], sdy[:, 1:2],
                                    AL.mult)
            nc.vector.tensor_tensor(sdy[:, 2:3], bn[:, 3:4], sdy[:, 2:3],
                                    AL.subtract)                     # cy
            nc.vector.tensor_scalar(sdy[:, 3:4], sdy[:, 1:2], 0.125, None,
                                    AL.mult)                         # ay/8
            ybn = spool.tile([H, BS], bf16, tag="ybn")
            nc.vector.tensor_scalar(ybn[:], yt[:], sdy[:, 3:4], sdy[:, 2:3],
                                    AL.mult, AL.add)

            # -------- out^T = relu(relu(W3^T@Ybn^T+b3) + r2t) --------
            psA = psaux.tile([128, BS], f32, tag="aux")
            nc.tensor.matmul(psA[:], lhsT=w3[:], rhs=ybn[:],
                             start=True, stop=True)
            r1t = spool.tile([128, BS], f32, tag="r1t")
            nc.scalar.activation(r1t[:], psA[:], FT.Relu, bias=bcol[:, 2:3])
            s = spool.tile([128, BS], f32, tag="s")
            nc.vector.tensor_tensor(s[:], r1t[:], r2t[:], AL.add)
            of = spool.tile([128, BS], f32, tag="of")
            nc.scalar.activation(of[:], s[:], FT.Relu)
            nc.sync.dma_start(OUTd[:], of[:])

    for inst, sem, val in deferred_waits:
        inst.wait_op(sem, val, "sem-ge")

    nc.compile()
    return nc


def _bn_sync(nc, dpool, spool, stl, bn, gcol, bcol, epsc):
    """AllGather per-core (H, W) stats [sum, sumsq, ...], reduce across the
    8 cores, compute affine coeffs a, c s.t. BN(x) = a*x + c (per-partition).
    Returns (a, c, global_sums)."""
    from concourse import mybir

    f32 = mybir.dt.float32
    FT = mybir.ActivationFunctionType
    AL = mybir.AluOpType

    W = stl.shape[1]
    src = dpool.tile([H, W], f32, tag=f"ccsrc{W}")
    dst = dpool.tile([N_CORES * H, W], f32, tag=f"ccdst{W}")
    nc.scalar.dma_start(src[:], stl[:])
    nc.gpsimd.collective_compute(
        "AllGather", AL.bypass, replica_groups=[list(range(N_CORES))],
        ins=[src.opt()], outs=[dst.opt()],
    )
    gath = spool.tile([H, 8 * W], f32, tag=f"gath{W}")
    nc.scalar.dma_start(
        gath[:].rearrange("k (c s) -> k c s", c=N_CORES),
        dst[:].rearrange("(c k) s -> k c s", c=N_CORES))
    # reduce over cores: layout (k, (c, s)) c-major slots
    r4 = spool.tile([H, 4 * W], f32, tag=f"r4{W}")
    nc.vector.tensor_tensor(r4[:], gath[:, 0:4 * W], gath[:, 4 * W:8 * W],
                            AL.add)
    r2 = spool.tile([H, 2 * W], f32, tag=f"r2s{W}")
    nc.vector.tensor_tensor(r2[:], r4[:, 0:2 * W], r4[:, 2 * W:4 * W], AL.add)
    st = spool.tile([H, W], f32, tag=f"stg{W}")
    nc.vector.tensor_tensor(st[:], r2[:, 0:W], r2[:, W:2 * W], AL.add)

    cf = spool.tile([H, 8], f32, tag="cf")
    me2 = cf[:, 0:2]   # [mean, E[x^2]]
    m = cf[:, 0:1]
    ex2 = cf[:, 1:2]
    v = cf[:, 2:3]
    sd = cf[:, 3:4]
    rinv = cf[:, 4:5]
    a = cf[:, 5:6]
    t = cf[:, 6:7]
    c = cf[:, 7:8]
    nc.vector.tensor_scalar(me2, st[:, 0:2], 1.0 / B, None, AL.mult)
    msq = spool.tile([H, 1], f32, tag="msq")
    nc.vector.tensor_tensor(msq[:], m, m, AL.mult)
    nc.vector.tensor_scalar(v, msq[:], -1.0, ex2, AL.mult, AL.add)
    nc.scalar.activation(sd, v, FT.Sqrt, bias=epsc[:])
    nc.vector.reciprocal(rinv, sd)
    nc.vector.tensor_tensor(a, rinv, bn[:, gcol:gcol + 1], AL.mult)
    nc.vector.tensor_tensor(t, m, a, AL.mult)
    nc.vector.tensor_tensor(c, bn[:, bcol:bcol + 1], t, AL.subtract)
    return a, c, st


def _prep_inputs(X, W1, b1, W2, b2, W3, b3, P, gz, bz, gy, by):
    bf = ml_dtypes.bfloat16
    per_core = []
    P_b = np.ascontiguousarray(P.reshape(RANK, H, QK)).astype(bf)
    W1_b = np.ascontiguousarray(W1.reshape(2, 128, H)).astype(bf)
    W2_b = np.ascontiguousarray(W2.reshape(2, 128, OUT)).astype(bf)
    W3_b = np.ascontiguousarray(W3).astype(bf)
    bc = np.zeros((128, 4), np.float32)
    bc[:, 0] = b1
    bc[:, 1] = b2
    bc[:, 2] = b3
    bnc = np.stack([gz, bz, gy, by], axis=1).astype(np.float32)
    ones = np.ones((1, 128), np.float32).astype(bf)
    for s in range(N_CORES):
        Xs = X[s * BS:(s + 1) * BS]
        XT = np.ascontiguousarray(Xs.T.reshape(2, 128, BS)).astype(bf)
        per_core.append({
            "XT": XT, "P": P_b, "W1": W1_b, "W2": W2_b, "W3": W3_b,
            "bcol": bc, "bn": bnc, "ones": ones,
        })
    return per_core


def kernel(**inputs):
    _ensure_axon_hooks_shim()
    from concourse.bass_utils import run_bass_kernel_spmd

    if "nc" not in _cache:
        _cache["nc"] = _build()
    nc = _cache["nc"]

    in_maps = _prep_inputs(**{k: np.asarray(v) for k, v in inputs.items()})
    res = run_bass_kernel_spmd(nc, in_maps, core_ids=list(range(N_CORES)))
    out = np.concatenate([m["out"].T for m in res.results], axis=0)
    return np.ascontiguousarray(out).astype(np.float32)


if __name__ == "__main__":
    import reference as R

    inputs = {k: np.asarray(v) for k, v in R.setup_inputs().items()}
    got = kernel(**inputs)
    exp = np.asarray(R.reference(**R.setup_inputs()))
    rel = np.linalg.norm(got - exp) / np.linalg.norm(exp)
    print("rel l2:", rel)


# revision 27
# speedup vs baseline: 1.3520x; 1.0737x over previous
"""Distributed Trainium2 kernel for nn_ApaBlock (8 NeuronCores, data-parallel).

Transposed dataflow (per core, batch shard of 256 rows), no PE transposes:
  Z^T = relu(W1^T @ X^T + b1)              [h, b] via lhsT=W1 (natural)
  W[p,(q,b)] = Z^T[q,b]                    partition-broadcast table, built
                                           once via broadcast-read DMAs
  scan over 8 ranks:
    M_q[p,b] = Zi^T[p,b] * Z^T[q,b]        DVE tensor_tensor bf16 SBUF (2x)
    G^T[k,b] += P_q^T @ M_q                128 accumulating matmuls (N=256)
    stats via ACT accum_out during PSUM evac; 1KB AllGather (sync-BN);
    Zi+1^T = a*G^T + c on ACT             (per-partition scale+bias)
  Y-BN via closed-form global sums piggybacked on the rank-7 sync.
  out^T = relu(relu(W3^T@Ybn^T+b3) + relu(W2^T@X^T+b2)); host transposes.

vs baseline: TensorE streaming work halved (no identity-matmul q-reduction),
DVE scale moved from 1x PSUM-sourced to 2x SBUF bf16, zero PE transposes.
"""

import os
import sys
import types

if "/opt/trn_rl_repo" not in sys.path:
    sys.path.insert(0, "/opt/trn_rl_repo")

import numpy as np
import ml_dtypes

N_CORES = 8
B, IN, H, OUT, RANK = 2048, 256, 128, 128, 8
BS = B // N_CORES  # 256 rows per core
EPS = 1e-5
QK = H * H  # 16384
NQC = 16            # max q values per M chunk
MCH = [4, 12] + [16] * 7   # chunk sizes (small first chunk: PE lead-in)
PCH = 2             # P dma split per rank (2MB chunks)

_cache = {}


def _ensure_axon_hooks_shim():
    """bass_utils imports antenv.axon_hooks when BASS_TRACE is set; provide a
    null shim so tracing degrades gracefully if the module is absent."""
    try:
        import antenv.axon_hooks  # noqa: F401
        return
    except ImportError:
        pass
    try:
        import antenv  # noqa: F401
    except ImportError:
        return
    mod = types.ModuleType("antenv.axon_hooks")
    _state = {"hook": None}
    mod.set_axon_ntff_profile_hook = lambda h: _state.__setitem__("hook", h)
    mod.get_axon_ntff_profile_hook = lambda: _state["hook"]
    sys.modules["antenv.axon_hooks"] = mod


def _build():
    from concourse import bacc, mybir, tile

    f32 = mybir.dt.float32
    bf16 = mybir.dt.bfloat16
    FT = mybir.ActivationFunctionType
    AL = mybir.AluOpType

    nc = bacc.Bacc("TRN2", target_bir_lowering=False, debug=False,
                   num_devices=N_CORES)

    XTd = nc.declare_dram_parameter("XT", [2, 128, BS], bf16, isOutput=False)
    Pd = nc.declare_dram_parameter("P", [RANK, H, QK], bf16, isOutput=False)
    W1d = nc.declare_dram_parameter("W1", [2, 128, H], bf16, isOutput=False)
    W2d = nc.declare_dram_parameter("W2", [2, 128, OUT], bf16, isOutput=False)
    W3d = nc.declare_dram_parameter("W3", [H, OUT], bf16, isOutput=False)
    BCd = nc.declare_dram_parameter("bcol", [128, 4], f32, isOutput=False)
    BNd = nc.declare_dram_parameter("bn", [H, 4], f32, isOutput=False)
    OUTd = nc.declare_dram_parameter("out", [OUT, BS], f32, isOutput=True)

    rg = [list(range(N_CORES))]
    deferred_waits = []  # (inst, sem, val): applied post-scheduling; the
    # single-core scheduling sim cannot model cross-core sem increments.

    with tile.TileContext(nc) as tc:
        with (
            tc.tile_pool(name="const", bufs=1) as cpool,
            tc.tile_pool(name="wtab", bufs=1) as wpool,
            tc.tile_pool(name="ppool", bufs=3) as ppool,
            tc.tile_pool(name="mpool", bufs=2) as mpool,
            tc.tile_pool(name="zit", bufs=2) as zitpool,
            tc.tile_pool(name="small", bufs=4) as spool,
            tc.tile_pool(name="gpool", bufs=2) as gpool,
            tc.tile_pool(name="psmain", bufs=2, space="PSUM") as psmain,
            tc.tile_pool(name="psaux", bufs=2, space="PSUM") as psaux,
            tc.tile_pool(name="dram", bufs=4, space="DRAM") as dpool,
        ):
            # ---------------- constants ----------------
            xt = cpool.tile([128, 2 * BS], bf16, tag="xt")
            for c in range(2):
                nc.sync.dma_start(xt[:, c * BS:(c + 1) * BS], XTd[c])
            w1 = cpool.tile([128, 2 * H], bf16, tag="w1")
            w2 = cpool.tile([128, 2 * OUT], bf16, tag="w2")
            for c in range(2):
                nc.sync.dma_start(w1[:, c * H:(c + 1) * H], W1d[c])
                nc.sync.dma_start(w2[:, c * OUT:(c + 1) * OUT], W2d[c])
            w3 = cpool.tile([H, OUT], bf16, tag="w3")
            nc.sync.dma_start(w3[:], W3d[:])
            bcol = cpool.tile([128, 4], f32, tag="bcol")
            nc.sync.dma_start(bcol[:], BCd[:])
            bn = cpool.tile([H, 4], f32, tag="bn")
            nc.sync.dma_start(bn[:], BNd[:])
            epsc = cpool.tile([H, 1], f32, tag="epsc")
            nc.vector.memset(epsc[:], EPS)
            yt = cpool.tile([H, BS], f32, tag="yt")
            nc.vector.memset(yt[:], 0.0)

            # Early dummy collective: absorbs cross-core launch skew.
            dsrc = dpool.tile([H, 2], f32, tag="ccsrc2")
            ddst = dpool.tile([N_CORES * H, 2], f32, tag="ccdst2")
            nc.sync.dma_start(dsrc[:], bn[:, 0:2])
            nc.gpsimd.collective_compute(
                "AllGather", AL.bypass, replica_groups=rg,
                ins=[dsrc.opt()], outs=[ddst.opt()],
            )
            # Early Sqrt: preload the ACT table set before the first sync.
            sqd = spool.tile([H, 1], f32, tag="sqd")
            nc.scalar.activation(sqd[:], epsc[:], FT.Sqrt)

            # ---------------- Z^T = relu(W1^T @ X^T + b1) ----------------
            psz = psaux.tile([128, BS], f32, tag="aux")
            for c in range(2):
                nc.tensor.matmul(
                    psz[:], lhsT=w1[:, c * H:(c + 1) * H],
                    rhs=xt[:, c * BS:(c + 1) * BS],
                    start=(c == 0), stop=(c == 1))
            zt = cpool.tile([128, BS], bf16, tag="zt")
            nc.scalar.activation(zt[:], psz[:], FT.Relu, bias=bcol[:, 0:1])

            # r2t = relu(W2^T @ X^T + b2)  (independent of the scan)
            psr2 = psaux.tile([128, BS], f32, tag="aux")
            for c in range(2):
                nc.tensor.matmul(
                    psr2[:], lhsT=w2[:, c * OUT:(c + 1) * OUT],
                    rhs=xt[:, c * BS:(c + 1) * BS],
                    start=(c == 0), stop=(c == 1))
            r2t = cpool.tile([128, BS], bf16, tag="r2t")
            nc.scalar.activation(r2t[:], psr2[:], FT.Relu, bias=bcol[:, 1:2])

            # ------- W[p, (q, b)] = Z^T[q, b]: partition-broadcast table ----
            # Flatten Z^T into DRAM, then replicate it across all 128
            # partitions with broadcast-read DMAs (stride-0 partition AP),
            # interleaved with the rank-0 P chunks on the same queue.
            zfd = dpool.tile([1, H * BS], bf16, tag="zflat")
            nc.scalar.dma_start(
                zfd[:].rearrange("o (q b) -> (o q) b", q=H), zt[:])
            wtab = wpool.tile([128, H * BS], bf16, tag="wtab")
            NWB = H * BS // 4
            p_sb0 = ppool.tile([128, QK], bf16, tag="p")
            for j in range(4):
                nc.sync.dma_start(
                    wtab[:, j * NWB:(j + 1) * NWB],
                    zfd[0:1, j * NWB:(j + 1) * NWB].broadcast_to((128, NWB)))
                if j < PCH:
                    w = QK // PCH
                    nc.sync.dma_start(p_sb0[:, j * w:(j + 1) * w],
                                      Pd[0][:, j * w:(j + 1) * w])

            # ---------------- scan over ranks ----------------
            zit = zt
            for r in range(RANK):
                if r == 0:
                    p_sb = p_sb0
                else:
                    p_sb = ppool.tile([128, QK], bf16, tag="p")
                    for pc in range(PCH):
                        w = QK // PCH
                        nc.sync.dma_start(p_sb[:, pc * w:(pc + 1) * w],
                                          Pd[r][:, pc * w:(pc + 1) * w])

                last = (r == RANK - 1)
                stw = 8 if last else 2
                stl = spool.tile([H, stw], f32, tag=f"stl{stw}")

                ps = psmain.tile([128, BS], f32, tag="mm")
                q0 = 0
                for nq in MCH:
                    m = mpool.tile([128, NQC * BS], bf16, tag="m")
                    nc.vector.tensor_tensor(
                        m[:, 0:nq * BS].rearrange("p (a b) -> p a b", b=BS),
                        wtab[:, q0 * BS:(q0 + nq) * BS].rearrange(
                            "p (a b) -> p a b", b=BS),
                        zit[:].rearrange("p (o b) -> p o b", o=1
                                         ).broadcast_to((128, nq, BS)),
                        AL.mult)
                    for i in range(nq):
                        q = q0 + i
                        nc.tensor.matmul(
                            ps[:], lhsT=p_sb[:, q * H:(q + 1) * H],
                            rhs=m[:, i * BS:(i + 1) * BS],
                            start=(q == 0), stop=(q == H - 1))
                    q0 += nq

                # stats + evac:  ghat = G^T (bf16), stl = [S1, S2, ...]
                ghat = gpool.tile([H, BS], bf16, tag="ghat")
                scr = gpool.tile([H, BS], bf16, tag="scr")
                nc.scalar.activation(ghat[:], ps[:], FT.Copy,
                                     accum_out=stl[:, 0:1])
                nc.scalar.activation(scr[:], ps[:], FT.Square,
                                     accum_out=stl[:, 1:2])
                if last:
                    # Piggyback Y-BN inputs on the final sync: with
                    # R = sum_{r<7} Zi (= yt now) and Zi_8 = a*G + c,
                    # SumY and SumY^2 expand in closed form from
                    # [S1G, S2G, S1R, S2R, SX] -- no 9th sync.
                    nc.scalar.activation(scr[:], yt[:], FT.Copy,
                                         accum_out=stl[:, 2:3])
                    nc.scalar.activation(scr[:], yt[:], FT.Square,
                                         accum_out=stl[:, 3:4])
                    scry2 = gpool.tile([H, BS], bf16, tag="scry2")
                    nc.vector.tensor_tensor(scry2[:], yt[:], ghat[:], AL.mult)
                    nc.scalar.activation(scr[:], scry2[:], FT.Copy,
                                         accum_out=stl[:, 4:5])

                # ---- cross-core AllGather of stats ----
                a_ap, c_ap, stg = _bn_sync(nc, dpool, spool, stl, bn,
                                           gcol=0, bcol=1, epsc=epsc)

                # Zi+1^T = a*G^T + c  (ACT per-partition scale+bias)
                zit_next = zitpool.tile([H, BS], bf16, tag="zit")
                nc.scalar.activation(zit_next[:], ghat[:], FT.Identity,
                                     scale=a_ap, bias=c_ap)
                nc.vector.tensor_tensor(yt[:], yt[:], zit_next[:], AL.add)
                zit = zit_next

            # ------- Y BN from closed-form global sums (no extra sync) ----
            # stg (global): [S1G, S2G, S1R, S2R, SX]; a_ap/c_ap = rank-7 BN.
            # SumY  = (S1R + a*S1G + B*c) / 8
            # SumY2 = (S2R + 2*(a*SX + c*S1R)
            #          + a^2*S2G + 2*a*c*S1G + B*c^2) / 64
            S1G, S2G = stg[:, 0:1], stg[:, 1:2]
            S1R, S2R = stg[:, 2:3], stg[:, 3:4]
            SX = stg[:, 4:5]
            w = spool.tile([H, 10], f32, tag="ywork")
            nc.vector.tensor_tensor(w[:, 0:1], a_ap, S1G, AL.mult)   # a*S1G
            nc.vector.tensor_scalar(w[:, 1:2], c_ap, float(B), w[:, 0:1],
                                    AL.mult, AL.add)                 # S1Z
            nc.vector.tensor_tensor(w[:, 2:3], w[:, 1:2], S1R, AL.add)  # SumY*8
            nc.vector.tensor_tensor(w[:, 3:4], a_ap, SX, AL.mult)
            nc.vector.tensor_tensor(w[:, 4:5], c_ap, S1R, AL.mult)
            nc.vector.tensor_tensor(w[:, 3:4], w[:, 3:4], w[:, 4:5], AL.add)
            # w3 = SRZ = a*SX + c*S1R
            nc.vector.tensor_tensor(w[:, 5:6], a_ap, a_ap, AL.mult)  # a^2
            nc.vector.tensor_tensor(w[:, 5:6], w[:, 5:6], S2G, AL.mult)
            nc.vector.tensor_tensor(w[:, 6:7], a_ap, c_ap, AL.mult)  # a*c
            nc.vector.tensor_tensor(w[:, 6:7], w[:, 6:7], S1G, AL.mult)
            nc.vector.tensor_tensor(w[:, 7:8], c_ap, c_ap, AL.mult)  # c^2
            nc.vector.tensor_scalar(w[:, 7:8], w[:, 7:8], float(B), None,
                                    AL.mult)
            # S2Z = a^2*S2G + 2*a*c*S1G + B*c^2
            nc.vector.tensor_scalar(w[:, 6:7], w[:, 6:7], 2.0, None, AL.mult)
            nc.vector.tensor_tensor(w[:, 5:6], w[:, 5:6], w[:, 6:7], AL.add)
            nc.vector.tensor_tensor(w[:, 5:6], w[:, 5:6], w[:, 7:8], AL.add)
            nc.vector.tensor_scalar(w[:, 3:4], w[:, 3:4], 2.0, None, AL.mult)
            nc.vector.tensor_tensor(w[:, 8:9], S2R, w[:, 3:4], AL.add)
            nc.vector.tensor_tensor(w[:, 8:9], w[:, 8:9], w[:, 5:6], AL.add)
            # w8 = SumY2*64;  mean/var of Y:
            nc.vector.tensor_scalar(w[:, 2:3], w[:, 2:3], 1.0 / (8.0 * B),
                                    None, AL.mult)                   # mY
            nc.vector.tensor_scalar(w[:, 8:9], w[:, 8:9], 1.0 / (64.0 * B),
                                    None, AL.mult)                   # E[Y^2]
            nc.vector.tensor_tensor(w[:, 9:10], w[:, 2:3], w[:, 2:3], AL.mult)
            nc.vector.tensor_scalar(w[:, 9:10], w[:, 9:10], -1.0, w[:, 8:9],
                                    AL.mult, AL.add)                 # var
            sdy = spool.tile([H, 4], f32, tag="ycoef")
            nc.scalar.activation(sdy[:, 0:1], w[:, 9:10], FT.Sqrt,
                                 bias=epsc[:])
            nc.vector.reciprocal(sdy[:, 1:2], sdy[:, 0:1])
            nc.vector.tensor_tensor(sdy[:, 1:2], sdy[:, 1:2], bn[:, 2:3],
                                    AL.mult)                         # ay
            nc.vector.tensor_tensor(sdy[:, 2:3], w[:, 2:3], sdy[:, 1:2],
                                    AL.mult)
            nc.vector.tensor_tensor(sdy[:, 2:3], bn[:, 3:4], sdy[:, 2:3],
                                    AL.subtract)                     # cy
            nc.vector.tensor_scalar(sdy[:, 3:4], sdy[:, 1:2], 0.125, None,
                                    AL.mult)                         # ay/8
            ybn = spool.tile([H, BS], bf16, tag="ybn")
            nc.vector.tensor_scalar(ybn[:], yt[:], sdy[:, 3:4], sdy[:, 2:3],
                                    AL.mult, AL.add)

            # -------- out^T = relu(relu(W3^T@Ybn^T+b3) + r2t) --------
            psA = psaux.tile([128, BS], f32, tag="aux")
            nc.tensor.matmul(psA[:], lhsT=w3[:], rhs=ybn[:],
                             start=True, stop=True)
            r1t = spool.tile([128, BS], f32, tag="r1t")
            nc.scalar.activation(r1t[:], psA[:], FT.Relu, bias=bcol[:, 2:3])
            s = spool.tile([128, BS], f32, tag="s")
            nc.vector.tensor_tensor(s[:], r1t[:], r2t[:], AL.add)
            of = spool.tile([128, BS], f32, tag="of")
            nc.scalar.activation(of[:], s[:], FT.Relu)
            nc.sync.dma_start(OUTd[:], of[:])

    for inst, sem, val in deferred_waits:
        inst.wait_op(sem, val, "sem-ge")

    nc.compile()
    return nc


def _bn_sync(nc, dpool, spool, stl, bn, gcol, bcol, epsc):
    """AllGather per-core (H, W) stats [sum, sumsq, ...], reduce across the
    8 cores, compute affine coeffs a, c s.t. BN(x) = a*x + c (per-partition).
    Returns (a, c, global_sums)."""
    from concourse import mybir

    f32 = mybir.dt.float32
    FT = mybir.ActivationFunctionType
    AL = mybir.AluOpType

    W = stl.shape[1]
    src = dpool.tile([H, W], f32, tag=f"ccsrc{W}")
    dst = dpool.tile([N_CORES * H, W], f32, tag=f"ccdst{W}")
    nc.scalar.dma_start(src[:], stl[:])
    nc.gpsimd.collective_compute(
        "AllGather", AL.bypass, replica_groups=[list(range(N_CORES))],
        ins=[src.opt()], outs=[dst.opt()],
    )
    gath = spool.tile([H, 8 * W], f32, tag=f"gath{W}")
    nc.scalar.dma_start(
        gath[:].rearrange("k (c s) -> k c s", c=N_CORES),
        dst[:].rearrange("(c k) s -> k c s", c=N_CORES))
    # reduce over cores: layout (k, (c, s)) c-major slots
    r4 = spool.tile([H, 4 * W], f32, tag=f"r4{W}")
    nc.vector.tensor_tensor(r4[:], gath[:, 0:4 * W], gath[:, 4 * W:8 * W],
                            AL.add)
    r2 = spool.tile([H, 2 * W], f32, tag=f"r2s{W}")
    nc.vector.tensor_tensor(r2[:], r4[:, 0:2 * W], r4[:, 2 * W:4 * W], AL.add)
    st = spool.tile([H, W], f32, tag=f"stg{W}")
    nc.vector.tensor_tensor(st[:], r2[:, 0:W], r2[:, W:2 * W], AL.add)

    cf = spool.tile([H, 8], f32, tag="cf")
    me2 = cf[:, 0:2]   # [mean, E[x^2]]
    m = cf[:, 0:1]
    ex2 = cf[:, 1:2]
    v = cf[:, 2:3]
    sd = cf[:, 3:4]
    rinv = cf[:, 4:5]
    a = cf[:, 5:6]
    t = cf[:, 6:7]
    c = cf[:, 7:8]
    nc.vector.tensor_scalar(me2, st[:, 0:2], 1.0 / B, None, AL.mult)
    msq = spool.tile([H, 1], f32, tag="msq")
    nc.vector.tensor_tensor(msq[:], m, m, AL.mult)
    nc.vector.tensor_scalar(v, msq[:], -1.0, ex2, AL.mult, AL.add)
    nc.scalar.activation(sd, v, FT.Sqrt, bias=epsc[:])
    nc.vector.reciprocal(rinv, sd)
    nc.vector.tensor_tensor(a, rinv, bn[:, gcol:gcol + 1], AL.mult)
    nc.vector.tensor_tensor(t, m, a, AL.mult)
    nc.vector.tensor_tensor(c, bn[:, bcol:bcol + 1], t, AL.subtract)
    return a, c, st


def _prep_inputs(X, W1, b1, W2, b2, W3, b3, P, gz, bz, gy, by):
    bf = ml_dtypes.bfloat16
    per_core = []
    P_b = np.ascontiguousarray(P.reshape(RANK, H, QK)).astype(bf)
    W1_b = np.ascontiguousarray(W1.reshape(2, 128, H)).astype(bf)
    W2_b = np.ascontiguousarray(W2.reshape(2, 128, OUT)).astype(bf)
    W3_b = np.ascontiguousarray(W3).astype(bf)
    bc = np.zeros((128, 4), np.float32)
    bc[:, 0] = b1
    bc[:, 1] = b2
    bc[:, 2] = b3
    bnc = np.stack([gz, bz, gy, by], axis=1).astype(np.float32)
    for s in range(N_CORES):
        Xs = X[s * BS:(s + 1) * BS]
        XT = np.ascontiguousarray(Xs.T.reshape(2, 128, BS)).astype(bf)
        per_core.append({
            "XT": XT, "P": P_b, "W1": W1_b, "W2": W2_b, "W3": W3_b,
            "bcol": bc, "bn": bnc,
        })
    return per_core


def kernel(**inputs):
    _ensure_axon_hooks_shim()
    from concourse.bass_utils import run_bass_kernel_spmd

    if "nc" not in _cache:
        _cache["nc"] = _build()
    nc = _cache["nc"]

    in_maps = _prep_inputs(**{k: np.asarray(v) for k, v in inputs.items()})
    res = run_bass_kernel_spmd(nc, in_maps, core_ids=list(range(N_CORES)))
    out = np.concatenate([m["out"].T for m in res.results], axis=0)
    return np.ascontiguousarray(out).astype(np.float32)


if __name__ == "__main__":
    import reference as R

    inputs = {k: np.asarray(v) for k, v in R.setup_inputs().items()}
    got = kernel(**inputs)
    exp = np.asarray(R.reference(**R.setup_inputs()))
    rel = np.linalg.norm(got - exp) / np.linalg.norm(exp)
    print("rel l2:", rel)


# revision 29
# speedup vs baseline: 1.4066x; 1.0404x over previous
"""Distributed Trainium2 kernel for nn_ApaBlock (8 NeuronCores, data-parallel).

Transposed dataflow (per core, batch shard of 256 rows), no PE transposes:
  Z^T = relu(W1^T @ X^T + b1)              [h, b] via lhsT=W1 (natural)
  W[p,(q,b)] = Z^T[q,b]                    partition-broadcast table, built
                                           once via broadcast-read DMAs
  scan over 8 ranks:
    M_q[p,b] = Zi^T[p,b] * Z^T[q,b]        DVE tensor_tensor bf16 SBUF (2x)
    G^T[k,b] += P_q^T @ M_q                128 accumulating matmuls (N=256)
    stats via ACT accum_out during PSUM evac; 1KB AllGather (sync-BN);
    Zi+1^T = a*G^T + c on ACT             (per-partition scale+bias)
  Y-BN via closed-form global sums piggybacked on the rank-7 sync.
  out^T = relu(relu(W3^T@Ybn^T+b3) + relu(W2^T@X^T+b2)); host transposes.

vs baseline: TensorE streaming work halved (no identity-matmul q-reduction),
DVE scale moved from 1x PSUM-sourced to 2x SBUF bf16, zero PE transposes.
"""

import os
import sys
import types

if "/opt/trn_rl_repo" not in sys.path:
    sys.path.insert(0, "/opt/trn_rl_repo")

import numpy as np
import ml_dtypes

N_CORES = 8
B, IN, H, OUT, RANK = 2048, 256, 128, 128, 8
BS = B // N_CORES  # 256 rows per core
EPS = 1e-5
QK = H * H  # 16384
NQC = 16            # max q values per M chunk
MCH = [4, 12] + [16] * 7   # chunk sizes (small first chunk: PE lead-in)
PCH = 2             # P dma split per rank (2MB chunks)

_cache = {}


def _ensure_axon_hooks_shim():
    """bass_utils imports antenv.axon_hooks when BASS_TRACE is set; provide a
    null shim so tracing degrades gracefully if the module is absent."""
    try:
        import antenv.axon_hooks  # noqa: F401
        return
    except ImportError:
        pass
    try:
        import antenv  # noqa: F401
    except ImportError:
        return
    mod = types.ModuleType("antenv.axon_hooks")
    _state = {"hook": None}
    mod.set_axon_ntff_profile_hook = lambda h: _state.__setitem__("hook", h)
    mod.get_axon_ntff_profile_hook = lambda: _state["hook"]
    sys.modules["antenv.axon_hooks"] = mod


def _build():
    from concourse import bacc, mybir, tile

    f32 = mybir.dt.float32
    bf16 = mybir.dt.bfloat16
    FT = mybir.ActivationFunctionType
    AL = mybir.AluOpType

    nc = bacc.Bacc("TRN2", target_bir_lowering=False, debug=False,
                   num_devices=N_CORES)

    XTd = nc.declare_dram_parameter("XT", [2, 128, BS], bf16, isOutput=False)
    Pd = nc.declare_dram_parameter("P", [RANK, H, QK], bf16, isOutput=False)
    W1d = nc.declare_dram_parameter("W1", [2, 128, H], bf16, isOutput=False)
    W2d = nc.declare_dram_parameter("W2", [2, 128, OUT], bf16, isOutput=False)
    W3d = nc.declare_dram_parameter("W3", [H, OUT], bf16, isOutput=False)
    BCd = nc.declare_dram_parameter("bcol", [128, 4], f32, isOutput=False)
    BNd = nc.declare_dram_parameter("bn", [H, 4], f32, isOutput=False)
    OUTd = nc.declare_dram_parameter("out", [OUT, BS], f32, isOutput=True)

    rg = [list(range(N_CORES))]
    deferred_waits = []  # (inst, sem, val): applied post-scheduling; the
    # single-core scheduling sim cannot model cross-core sem increments.

    with tile.TileContext(nc) as tc:
        with (
            tc.tile_pool(name="const", bufs=1) as cpool,
            tc.tile_pool(name="wtab", bufs=1) as wpool,
            tc.tile_pool(name="ppool", bufs=4) as ppool,
            tc.tile_pool(name="mpool", bufs=3) as mpool,
            tc.tile_pool(name="zit", bufs=2) as zitpool,
            tc.tile_pool(name="small", bufs=4) as spool,
            tc.tile_pool(name="gpool", bufs=2) as gpool,
            tc.tile_pool(name="psmain", bufs=2, space="PSUM") as psmain,
            tc.tile_pool(name="psaux", bufs=2, space="PSUM") as psaux,
            tc.tile_pool(name="dram", bufs=4, space="DRAM") as dpool,
        ):
            # ---------------- constants ----------------
            xt = cpool.tile([128, 2 * BS], bf16, tag="xt")
            for c in range(2):
                nc.sync.dma_start(xt[:, c * BS:(c + 1) * BS], XTd[c])
            w1 = cpool.tile([128, 2 * H], bf16, tag="w1")
            w2 = cpool.tile([128, 2 * OUT], bf16, tag="w2")
            for c in range(2):
                nc.sync.dma_start(w1[:, c * H:(c + 1) * H], W1d[c])
                nc.sync.dma_start(w2[:, c * OUT:(c + 1) * OUT], W2d[c])
            w3 = cpool.tile([H, OUT], bf16, tag="w3")
            nc.sync.dma_start(w3[:], W3d[:])
            bcol = cpool.tile([128, 4], f32, tag="bcol")
            nc.sync.dma_start(bcol[:], BCd[:])
            bn = cpool.tile([H, 4], f32, tag="bn")
            nc.sync.dma_start(bn[:], BNd[:])
            epsc = cpool.tile([H, 1], f32, tag="epsc")
            nc.vector.memset(epsc[:], EPS)
            yt = cpool.tile([H, BS], f32, tag="yt")
            nc.vector.memset(yt[:], 0.0)

            # Early dummy collective: absorbs cross-core launch skew.
            dsrc = dpool.tile([H, 2], f32, tag="ccsrc2")
            ddst = dpool.tile([N_CORES * H, 2], f32, tag="ccdst2")
            nc.sync.dma_start(dsrc[:], bn[:, 0:2])
            nc.gpsimd.collective_compute(
                "AllGather", AL.bypass, replica_groups=rg,
                ins=[dsrc.opt()], outs=[ddst.opt()],
            )
            # Early Sqrt: preload the ACT table set before the first sync.
            sqd = spool.tile([H, 1], f32, tag="sqd")
            nc.scalar.activation(sqd[:], epsc[:], FT.Sqrt)

            # ---------------- Z^T = relu(W1^T @ X^T + b1) ----------------
            psz = psaux.tile([128, BS], f32, tag="aux")
            for c in range(2):
                nc.tensor.matmul(
                    psz[:], lhsT=w1[:, c * H:(c + 1) * H],
                    rhs=xt[:, c * BS:(c + 1) * BS],
                    start=(c == 0), stop=(c == 1))
            zt = cpool.tile([128, BS], bf16, tag="zt")
            nc.scalar.activation(zt[:], psz[:], FT.Relu, bias=bcol[:, 0:1])

            # r2t = relu(W2^T @ X^T + b2)  (independent of the scan)
            psr2 = psaux.tile([128, BS], f32, tag="aux")
            for c in range(2):
                nc.tensor.matmul(
                    psr2[:], lhsT=w2[:, c * OUT:(c + 1) * OUT],
                    rhs=xt[:, c * BS:(c + 1) * BS],
                    start=(c == 0), stop=(c == 1))
            r2t = cpool.tile([128, BS], bf16, tag="r2t")
            nc.scalar.activation(r2t[:], psr2[:], FT.Relu, bias=bcol[:, 1:2])

            # ------- W[p, (q, b)] = Z^T[q, b]: partition-broadcast table ----
            # Flatten Z^T into DRAM, then replicate it across all 128
            # partitions with broadcast-read DMAs (stride-0 partition AP),
            # interleaved with the rank-0 P chunks on the same queue.
            zfd = dpool.tile([1, H * BS], bf16, tag="zflat")
            nc.scalar.dma_start(
                zfd[:].rearrange("o (q b) -> (o q) b", q=H), zt[:])
            wtab = wpool.tile([128, H * BS], bf16, tag="wtab")
            NWB = H * BS // 4
            HQK = QK // 2
            p_sb0a = ppool.tile([128, HQK], bf16, tag="p")
            p_sb0b = ppool.tile([128, HQK], bf16, tag="p")
            p_sb0 = [p_sb0a, p_sb0b]
            for j in range(4):
                nc.sync.dma_start(
                    wtab[:, j * NWB:(j + 1) * NWB],
                    zfd[0:1, j * NWB:(j + 1) * NWB].broadcast_to((128, NWB)))
                if j < 2:
                    nc.sync.dma_start(p_sb0[j][:], Pd[0][:, j * HQK:(j + 1) * HQK])

            # ---------------- scan over ranks ----------------
            zit = zt
            for r in range(RANK):
                if r == 0:
                    p_sb = p_sb0
                else:
                    p_sba = ppool.tile([128, HQK], bf16, tag="p")
                    p_sbb = ppool.tile([128, HQK], bf16, tag="p")
                    p_sb = [p_sba, p_sbb]
                    for h in range(2):
                        nc.sync.dma_start(p_sb[h][:],
                                          Pd[r][:, h * HQK:(h + 1) * HQK])

                last = (r == RANK - 1)
                stw = 8 if last else 2
                stl = spool.tile([H, stw], f32, tag=f"stl{stw}")

                ps = psmain.tile([128, BS], f32, tag="mm")
                q0 = 0
                for nq in MCH:
                    m = mpool.tile([128, NQC * BS], bf16, tag="m")
                    nc.vector.tensor_tensor(
                        m[:, 0:nq * BS].rearrange("p (a b) -> p a b", b=BS),
                        wtab[:, q0 * BS:(q0 + nq) * BS].rearrange(
                            "p (a b) -> p a b", b=BS),
                        zit[:].rearrange("p (o b) -> p o b", o=1
                                         ).broadcast_to((128, nq, BS)),
                        AL.mult)
                    for i in range(nq):
                        q = q0 + i
                        ph = p_sb[q // 64]
                        ql = q % 64
                        nc.tensor.matmul(
                            ps[:], lhsT=ph[:, ql * H:(ql + 1) * H],
                            rhs=m[:, i * BS:(i + 1) * BS],
                            start=(q == 0), stop=(q == H - 1))
                    q0 += nq

                # stats + evac:  ghat = G^T (bf16), stl = [S1, S2, ...]
                ghat = gpool.tile([H, BS], bf16, tag="ghat")
                scr = gpool.tile([H, BS], bf16, tag="scr")
                nc.scalar.activation(ghat[:], ps[:], FT.Copy,
                                     accum_out=stl[:, 0:1])
                nc.scalar.activation(scr[:], ps[:], FT.Square,
                                     accum_out=stl[:, 1:2])
                if last:
                    # Piggyback Y-BN inputs on the final sync: with
                    # R = sum_{r<7} Zi (= yt now) and Zi_8 = a*G + c,
                    # SumY and SumY^2 expand in closed form from
                    # [S1G, S2G, S1R, S2R, SX] -- no 9th sync.
                    nc.scalar.activation(scr[:], yt[:], FT.Copy,
                                         accum_out=stl[:, 2:3])
                    nc.scalar.activation(scr[:], yt[:], FT.Square,
                                         accum_out=stl[:, 3:4])
                    scry2 = gpool.tile([H, BS], bf16, tag="scry2")
                    nc.vector.tensor_tensor(scry2[:], yt[:], ghat[:], AL.mult)
                    nc.scalar.activation(scr[:], scry2[:], FT.Copy,
                                         accum_out=stl[:, 4:5])

                # ---- cross-core AllGather of stats ----
                a_ap, c_ap, stg = _bn_sync(nc, dpool, spool, stl, bn,
                                           gcol=0, bcol=1, epsc=epsc)

                # Zi+1^T = a*G^T + c  (ACT per-partition scale+bias)
                zit_next = zitpool.tile([H, BS], bf16, tag="zit")
                nc.scalar.activation(zit_next[:], ghat[:], FT.Identity,
                                     scale=a_ap, bias=c_ap)
                nc.vector.tensor_tensor(yt[:], yt[:], zit_next[:], AL.add)
                zit = zit_next

            # ------- Y BN from closed-form global sums (no extra sync) ----
            # stg (global): [S1G, S2G, S1R, S2R, SX]; a_ap/c_ap = rank-7 BN.
            # SumY  = (S1R + a*S1G + B*c) / 8
            # SumY2 = (S2R + 2*(a*SX + c*S1R)
            #          + a^2*S2G + 2*a*c*S1G + B*c^2) / 64
            S1G, S2G = stg[:, 0:1], stg[:, 1:2]
            S1R, S2R = stg[:, 2:3], stg[:, 3:4]
            SX = stg[:, 4:5]
            w = spool.tile([H, 10], f32, tag="ywork")
            nc.vector.tensor_tensor(w[:, 0:1], a_ap, S1G, AL.mult)   # a*S1G
            nc.vector.tensor_scalar(w[:, 1:2], c_ap, float(B), w[:, 0:1],
                                    AL.mult, AL.add)                 # S1Z
            nc.vector.tensor_tensor(w[:, 2:3], w[:, 1:2], S1R, AL.add)  # SumY*8
            nc.vector.tensor_tensor(w[:, 3:4], a_ap, SX, AL.mult)
            nc.vector.tensor_tensor(w[:, 4:5], c_ap, S1R, AL.mult)
            nc.vector.tensor_tensor(w[:, 3:4], w[:, 3:4], w[:, 4:5], AL.add)
            # w3 = SRZ = a*SX + c*S1R
            nc.vector.tensor_tensor(w[:, 5:6], a_ap, a_ap, AL.mult)  # a^2
            nc.vector.tensor_tensor(w[:, 5:6], w[:, 5:6], S2G, AL.mult)
            nc.vector.tensor_tensor(w[:, 6:7], a_ap, c_ap, AL.mult)  # a*c
            nc.vector.tensor_tensor(w[:, 6:7], w[:, 6:7], S1G, AL.mult)
            nc.vector.tensor_tensor(w[:, 7:8], c_ap, c_ap, AL.mult)  # c^2
            nc.vector.tensor_scalar(w[:, 7:8], w[:, 7:8], float(B), None,
                                    AL.mult)
            # S2Z = a^2*S2G + 2*a*c*S1G + B*c^2
            nc.vector.tensor_scalar(w[:, 6:7], w[:, 6:7], 2.0, None, AL.mult)
            nc.vector.tensor_tensor(w[:, 5:6], w[:, 5:6], w[:, 6:7], AL.add)
            nc.vector.tensor_tensor(w[:, 5:6], w[:, 5:6], w[:, 7:8], AL.add)
            nc.vector.tensor_scalar(w[:, 3:4], w[:, 3:4], 2.0, None, AL.mult)
            nc.vector.tensor_tensor(w[:, 8:9], S2R, w[:, 3:4], AL.add)
            nc.vector.tensor_tensor(w[:, 8:9], w[:, 8:9], w[:, 5:6], AL.add)
            # w8 = SumY2*64;  mean/var of Y:
            nc.vector.tensor_scalar(w[:, 2:3], w[:, 2:3], 1.0 / (8.0 * B),
                                    None, AL.mult)                   # mY
            nc.vector.tensor_scalar(w[:, 8:9], w[:, 8:9], 1.0 / (64.0 * B),
                                    None, AL.mult)                   # E[Y^2]
            nc.vector.tensor_tensor(w[:, 9:10], w[:, 2:3], w[:, 2:3], AL.mult)
            nc.vector.tensor_scalar(w[:, 9:10], w[:, 9:10], -1.0, w[:, 8:9],
                                    AL.mult, AL.add)                 # var
            sdy = spool.tile([H, 4], f32, tag="ycoef")
            nc.scalar.activation(sdy[:, 0:1], w[:, 9:10], FT.Sqrt,
                                 bias=epsc[:])
            nc.vector.reciprocal(sdy[:, 1:2], sdy[:, 0:1])
            nc.vector.tensor_tensor(sdy[:, 1:2], sdy[:, 1:2], bn[:, 2:3],
                                    AL.mult)                         # ay
            nc.vector.tensor_tensor(sdy[:, 2:3], w[:, 2:3], sdy[:, 1:2],
                                    AL.mult)
            nc.vector.tensor_tensor(sdy[:, 2:3], bn[:, 3:4], sdy[:, 2:3],
                                    AL.subtract)                     # cy
            nc.vector.tensor_scalar(sdy[:, 3:4], sdy[:, 1:2], 0.125, None,
                                    AL.mult)                         # ay/8
            ybn = spool.tile([H, BS], bf16, tag="ybn")
            nc.vector.tensor_scalar(ybn[:], yt[:], sdy[:, 3:4], sdy[:, 2:3],
                                    AL.mult, AL.add)

            # -------- out^T = relu(relu(W3^T@Ybn^T+b3) + r2t) --------
            psA = psaux.tile([128, BS], f32, tag="aux")
            nc.tensor.matmul(psA[:], lhsT=w3[:], rhs=ybn[:],
                             start=True, stop=True)
            r1t = spool.tile([128, BS], f32, tag="r1t")
            nc.scalar.activation(r1t[:], psA[:], FT.Relu, bias=bcol[:, 2:3])
            s = spool.tile([128, BS], f32, tag="s")
            nc.vector.tensor_tensor(s[:], r1t[:], r2t[:], AL.add)
            of = spool.tile([128, BS], f32, tag="of")
            nc.scalar.activation(of[:], s[:], FT.Relu)
            nc.sync.dma_start(OUTd[:], of[:])

    for inst, sem, val in deferred_waits:
        inst.wait_op(sem, val, "sem-ge")

    nc.compile()
    return nc


def _bn_sync(nc, dpool, spool, stl, bn, gcol, bcol, epsc):
    """AllGather per-core (H, W) stats [sum, sumsq, ...], reduce across the
    8 cores, compute affine coeffs a, c s.t. BN(x) = a*x + c (per-partition).
    Returns (a, c, global_sums)."""
    from concourse import mybir

    f32 = mybir.dt.float32
    FT = mybir.ActivationFunctionType
    AL = mybir.AluOpType

    W = stl.shape[1]
    src = dpool.tile([H, W], f32, tag=f"ccsrc{W}")
    dst = dpool.tile([N_CORES * H, W], f32, tag=f"ccdst{W}")
    nc.scalar.dma_start(src[:], stl[:])
    nc.gpsimd.collective_compute(
        "AllGather", AL.bypass, replica_groups=[list(range(N_CORES))],
        ins=[src.opt()], outs=[dst.opt()],
    )
    gath = spool.tile([H, 8 * W], f32, tag=f"gath{W}")
    nc.scalar.dma_start(
        gath[:].rearrange("k (c s) -> k c s", c=N_CORES),
        dst[:].rearrange("(c k) s -> k c s", c=N_CORES))
    # reduce over cores: layout (k, (c, s)) c-major slots
    r4 = spool.tile([H, 4 * W], f32, tag=f"r4{W}")
    nc.vector.tensor_tensor(r4[:], gath[:, 0:4 * W], gath[:, 4 * W:8 * W],
                            AL.add)
    r2 = spool.tile([H, 2 * W], f32, tag=f"r2s{W}")
    nc.vector.tensor_tensor(r2[:], r4[:, 0:2 * W], r4[:, 2 * W:4 * W], AL.add)
    st = spool.tile([H, W], f32, tag=f"stg{W}")
    nc.vector.tensor_tensor(st[:], r2[:, 0:W], r2[:, W:2 * W], AL.add)

    cf = spool.tile([H, 8], f32, tag="cf")
    me2 = cf[:, 0:2]   # [mean, E[x^2]]
    m = cf[:, 0:1]
    ex2 = cf[:, 1:2]
    v = cf[:, 2:3]
    sd = cf[:, 3:4]
    rinv = cf[:, 4:5]
    a = cf[:, 5:6]
    t = cf[:, 6:7]
    c = cf[:, 7:8]
    nc.vector.tensor_scalar(me2, st[:, 0:2], 1.0 / B, None, AL.mult)
    msq = spool.tile([H, 1], f32, tag="msq")
    nc.vector.tensor_tensor(msq[:], m, m, AL.mult)
    nc.vector.tensor_scalar(v, msq[:], -1.0, ex2, AL.mult, AL.add)
    nc.scalar.activation(sd, v, FT.Sqrt, bias=epsc[:])
    nc.vector.reciprocal(rinv, sd)
    nc.vector.tensor_tensor(a, rinv, bn[:, gcol:gcol + 1], AL.mult)
    nc.vector.tensor_tensor(t, m, a, AL.mult)
    nc.vector.tensor_tensor(c, bn[:, bcol:bcol + 1], t, AL.subtract)
    return a, c, st


def _prep_inputs(X, W1, b1, W2, b2, W3, b3, P, gz, bz, gy, by):
    bf = ml_dtypes.bfloat16
    per_core = []
    P_b = np.ascontiguousarray(P.reshape(RANK, H, QK)).astype(bf)
    W1_b = np.ascontiguousarray(W1.reshape(2, 128, H)).astype(bf)
    W2_b = np.ascontiguousarray(W2.reshape(2, 128, OUT)).astype(bf)
    W3_b = np.ascontiguousarray(W3).astype(bf)
    bc = np.zeros((128, 4), np.float32)
    bc[:, 0] = b1
    bc[:, 1] = b2
    bc[:, 2] = b3
    bnc = np.stack([gz, bz, gy, by], axis=1).astype(np.float32)
    for s in range(N_CORES):
        Xs = X[s * BS:(s + 1) * BS]
        XT = np.ascontiguousarray(Xs.T.reshape(2, 128, BS)).astype(bf)
        per_core.append({
            "XT": XT, "P": P_b, "W1": W1_b, "W2": W2_b, "W3": W3_b,
            "bcol": bc, "bn": bnc,
        })
    return per_core


def kernel(**inputs):
    _ensure_axon_hooks_shim()
    from concourse.bass_utils import run_bass_kernel_spmd

    if "nc" not in _cache:
        _cache["nc"] = _build()
    nc = _cache["nc"]

    in_maps = _prep_inputs(**{k: np.asarray(v) for k, v in inputs.items()})
    res = run_bass_kernel_spmd(nc, in_maps, core_ids=list(range(N_CORES)))
    out = np.concatenate([m["out"].T for m in res.results], axis=0)
    return np.ascontiguousarray(out).astype(np.float32)


if __name__ == "__main__":
    import reference as R

    inputs = {k: np.asarray(v) for k, v in R.setup_inputs().items()}
    got = kernel(**inputs)
    exp = np.asarray(R.reference(**R.setup_inputs()))
    rel = np.linalg.norm(got - exp) / np.linalg.norm(exp)
    print("rel l2:", rel)
